# revision 5
# baseline (speedup 1.0000x reference)
"""Trainium2 Bass kernel for the gnn_message_passing ConvLayer problem.

Computes, for feature i in 0..4 and rotation r in 0..15:
    g      = exp(-(rho-mu_r)^2/cr) * exp(-(mod(theta+rot_r)-mu_t)^2/ct) * mask
    g      = g / (sum_k g + eps)
    desc   = einsum('svk,sv->sk', g, feats_i)
    conv   = desc @ W_i + b_i
    out_i  = relu(max_r conv)            # [NS, K]
out = stack(out_i, axis=2).reshape(NS, K*F)

Sharding: data-parallel over the NS=512 sample axis across 8 NeuronCores
(64 samples/core); the tiny per-feature params are replicated. No cross-core
communication.

Fast path: setup_inputs() builds mu/sigma as a tiled 5x16 (rho x theta) grid
shared across features, so the Gaussian factorises:
    g[..,k] = g_rho[.., k//16] * g_th[.., k%16]
which cuts the exp count 25x and turns the vertex reduction into tiny
batched matmuls. This structure is detected at runtime (exact equality
checks on the host); arbitrary parameters fall back to a generic kernel.
"""

import math
import sys

import numpy as np

sys.path.insert(0, "/opt/trn_rl_repo")

import concourse.bacc as bacc
import concourse.bass as bass
import concourse.tile as tile
from concourse import mybir
from concourse.bass_utils import run_bass_kernel_spmd

F32 = mybir.dt.float32
AF = mybir.ActivationFunctionType
ALU = mybir.AluOpType
AX = mybir.AxisListType

N_CORES = 8
NS = 512
NV = 128          # vertices / patch == SBUF partition count
NF = 5            # features
K = 80            # gaussian kernels
R = 16            # rotations
NT = 16           # theta centers (fast path)
NR = 5            # rho centers (fast path)
SC = NS // N_CORES  # samples per core
EPS = 1e-5
TWO_PI = 2.0 * math.pi
RC = 4            # rotations per chunk (fast path)


def _bcast(ap, dim, shape):
    """Insert a stride-0 dim at `dim` and broadcast to `shape`."""
    return ap.unsqueeze(dim).to_broadcast(shape)


def _dma_bcast_rows(nc, out_sb, dram_handle, n_free):
    """DMA a [1, n_free] DRAM tensor into all partitions of out_sb."""
    src = dram_handle[:]
    ap = bass.AP(
        tensor=src.tensor,
        offset=src.offset,
        ap=[[0, out_sb.partition_size()], [1, n_free]],
    )
    nc.sync.dma_start(out=out_sb, in_=ap)


# ---------------------------------------------------------------------------
# Fast (factored) kernel
# ---------------------------------------------------------------------------

def build_factored(neg_inv_ct, neg_inv_cr):
    nc = bacc.Bacc(None, target_bir_lowering=False)

    x_d = nc.dram_tensor("x", [SC, NV, 8], F32, kind="ExternalInput")
    mu_t_d = nc.dram_tensor("mu_t", [1, NT], F32, kind="ExternalInput")
    mu_r_d = nc.dram_tensor("mu_r", [1, NR], F32, kind="ExternalInput")
    # W_re[i, kt, kr, :] = W_conv[i, kr*16+kt, :]  flattened to [NF, NT, NR*K]
    w_d = nc.dram_tensor("w_re", [NF, NT, NR * K], F32, kind="ExternalInput")
    b_d = nc.dram_tensor("b_c", [NF, K], F32, kind="ExternalInput")
    id_d = nc.dram_tensor("ident", [128, 128], F32, kind="ExternalInput")
    out_d = nc.dram_tensor("out", [SC, K * NF], F32, kind="ExternalOutput")

    with tile.TileContext(nc) as tc:
        with (
            tc.tile_pool(name="const", bufs=1) as const,
            tc.tile_pool(name="work", bufs=1) as work,
            tc.tile_pool(name="big", bufs=2) as big,
            tc.tile_pool(name="psum", bufs=2, space="PSUM") as psum,
        ):
            # ---- loads -------------------------------------------------
            x_sb = const.tile([SC, NV * 8], F32)
            nc.sync.dma_start(out=x_sb, in_=x_d[:].rearrange("s v c -> s (v c)"))
            ident = const.tile([128, 128], F32)
            nc.sync.dma_start(out=ident, in_=id_d[:])
            mu_tb = const.tile([128, NT], F32)
            _dma_bcast_rows(nc, mu_tb, mu_t_d, NT)
            mu_rb = const.tile([128, NR], F32)
            _dma_bcast_rows(nc, mu_rb, mu_r_d, NR)
            w_sb = const.tile([NT, NF, NR * K], F32)
            nc.sync.dma_start(out=w_sb, in_=w_d[:].rearrange("i t q -> t i q"))
            b_sb = const.tile([K, NF], F32)
            nc.sync.dma_start(out=b_sb, in_=b_d[:].rearrange("i k -> k i"))

            # ---- transpose per-channel to [NV, SC] ---------------------
            xv = x_sb.rearrange("s (v c) -> s v c", c=8)
            featsT = const.tile([NV, NF, SC], F32)
            rhoT = const.tile([NV, SC], F32)
            thetaT = const.tile([NV, SC], F32)
            maskT = const.tile([NV, SC], F32)
            targets = [featsT[:, i, :] for i in range(NF)] + [rhoT, thetaT, maskT]
            for c in range(8):
                pt = psum.tile([NV, SC], F32, tag="tpsum")
                nc.tensor.transpose(pt, xv[:, :, c], ident[:SC, :SC])
                nc.scalar.copy(targets[c], pt)

            # ---- T = mod(theta + rot, 2pi), all rotations --------------
            T = work.tile([NV, R, SC], F32)
            for r in range(R):
                nc.vector.tensor_scalar_add(T[:, r, :], thetaT, (TWO_PI / R) * r)
            ge = work.tile([NV, R, SC], F32)
            nc.vector.tensor_scalar(ge, T, TWO_PI, None, ALU.is_ge)
            nc.vector.scalar_tensor_tensor(T, ge, -TWO_PI, T, ALU.mult, ALU.add)

            # ---- rho gaussians [NV, SC, NR] ----------------------------
            grho = work.tile([NV, SC, NR], F32)
            nc.vector.tensor_sub(
                grho,
                _bcast(rhoT, 2, [NV, SC, NR]),
                _bcast(mu_rb, 1, [NV, SC, NR]),
            )
            nc.scalar.activation(grho, grho, AF.Square)
            nc.scalar.activation(grho, grho, AF.Exp, scale=float(neg_inv_cr))
            Sr = work.tile([NV, SC], F32)
            nc.vector.reduce_sum(Sr, grho, axis=AX.X)

            # mask * feats, per feature: [NV, NF, SC]
            mf = work.tile([NV, NF, SC], F32)
            nc.vector.tensor_mul(mf, featsT, _bcast(maskT, 1, [NV, NF, SC]))

            # running max over rotations of conv output: [K, NF, SC]
            convmax = const.tile([K, NF, SC], F32)

            n_chunks = R // RC
            PB = 16  # (r,s) pairs per PSUM bank in the desc stage
            for ch in range(n_chunks):
                r0 = ch * RC
                # gth[v, rc, s, kt]
                gth = big.tile([NV, RC, SC, NT], F32, tag="gth")
                nc.vector.tensor_sub(
                    gth,
                    _bcast(T[:, r0 : r0 + RC, :], 3, [NV, RC, SC, NT]),
                    _bcast(_bcast(mu_tb, 1, [NV, RC, NT]), 2, [NV, RC, SC, NT]),
                )
                nc.scalar.activation(gth, gth, AF.Square)
                nc.scalar.activation(gth, gth, AF.Exp, scale=float(neg_inv_ct))

                # denom = St * Sr + eps ; recip
                St = work.tile([NV, RC, SC], F32, tag="St")
                nc.vector.reduce_sum(St, gth, axis=AX.X)
                nc.vector.tensor_mul(St, St, _bcast(Sr, 1, [NV, RC, SC]))
                nc.vector.tensor_scalar_add(St, St, EPS)
                rec = work.tile([NV, RC, SC], F32, tag="rec")
                nc.vector.reciprocal(rec, St)

                # w[v, i, rc, s] then u[v, rc, s, i, kr]
                wgt = work.tile([NV, NF, RC, SC], F32, tag="wgt")
                nc.vector.tensor_mul(
                    wgt,
                    _bcast(mf, 2, [NV, NF, RC, SC]),
                    _bcast(rec, 1, [NV, NF, RC, SC]),
                )
                u = big.tile([NV, RC, SC, NF, NR], F32, tag="u")
                for i in range(NF):
                    nc.vector.tensor_mul(
                        u[:, :, :, i, :],
                        _bcast(wgt[:, i, :, :], 3, [NV, RC, SC, NR]),
                        _bcast(grho, 1, [NV, RC, SC, NR]),
                    )

                # desc matmuls: per (r,s): [NT x 25] = gth_rs^T @ u_rs
                npairs = RC * SC
                desc = big.tile([NT, npairs, NF, NR], F32, tag="desc")
                for b0 in range(0, npairs, PB):
                    nb = min(PB, npairs - b0)
                    dp = psum.tile([NT, PB * NF * NR], F32, tag="dpsum")
                    for p in range(nb):
                        pair = b0 + p
                        r, s = divmod(pair, SC)
                        nc.tensor.matmul(
                            dp[:, p * 25 : (p + 1) * 25],
                            gth[:, r, s, :],
                            u[:, r, s, :, :].rearrange("v i k -> v (i k)"),
                            start=True, stop=True,
                        )
                    nc.scalar.copy(
                        desc[:, b0 : b0 + nb, :, :].rearrange("t p i k -> t (p i k)"),
                        dp[:, : nb * 25],
                    )

                # conv: per feature, accumulate over kr; then max over r
                for i in range(NF):
                    cp = psum.tile([K, npairs], F32, tag="cpsum")
                    for kr in range(NR):
                        nc.tensor.matmul(
                            cp,
                            w_sb[:, i, kr * K : (kr + 1) * K],
                            desc[:, :, i, kr],
                            start=(kr == 0), stop=(kr == NR - 1),
                        )
                    red = work.tile([K, SC], F32, tag="red")
                    nc.vector.reduce_max(
                        red, cp.rearrange("k (r s) -> k s r", r=RC), axis=AX.X
                    )
                    if ch == 0:
                        nc.vector.tensor_copy(convmax[:, i, :], red)
                    else:
                        nc.vector.tensor_max(convmax[:, i, :], convmax[:, i, :], red)

            # ---- tail: bias + relu, transpose to [SC, K], interleave ---
            out_sb = const.tile([SC, K * NF], F32)
            oview = out_sb.rearrange("s (k i) -> s k i", i=NF)
            for i in range(NF):
                act = work.tile([K, SC], F32, tag="act")
                nc.scalar.activation(
                    act, convmax[:, i, :], AF.Relu, bias=b_sb[:, i : i + 1]
                )
                pt = psum.tile([SC, K], F32, tag="opsum")
                nc.tensor.transpose(pt, act, ident[:K, :K])
                nc.vector.tensor_copy(oview[:, :, i], pt)
            nc.sync.dma_start(out=out_d[:], in_=out_sb)

    return nc


# ---------------------------------------------------------------------------
# Generic fallback kernel (arbitrary mu/sigma): correct, slower
# ---------------------------------------------------------------------------

def build_generic():
    nc = bacc.Bacc(None, target_bir_lowering=False)

    x_d = nc.dram_tensor("x", [SC, NV, 8], F32, kind="ExternalInput")
    # params broadcast-ready, flattened [1, NF*K]
    mu_t_d = nc.dram_tensor("mu_t", [1, NF * K], F32, kind="ExternalInput")
    nict_d = nc.dram_tensor("nict", [1, NF * K], F32, kind="ExternalInput")
    mu_r_d = nc.dram_tensor("mu_r", [1, NF * K], F32, kind="ExternalInput")
    nicr_d = nc.dram_tensor("nicr", [1, NF * K], F32, kind="ExternalInput")
    w_d = nc.dram_tensor("w_c", [NF, K, K], F32, kind="ExternalInput")
    b_d = nc.dram_tensor("b_c", [NF, K], F32, kind="ExternalInput")
    id_d = nc.dram_tensor("ident", [128, 128], F32, kind="ExternalInput")
    out_d = nc.dram_tensor("out", [SC, K * NF], F32, kind="ExternalOutput")

    with tile.TileContext(nc) as tc:
        with (
            tc.tile_pool(name="const", bufs=1) as const,
            tc.tile_pool(name="work", bufs=1) as work,
            tc.tile_pool(name="big", bufs=2) as big,
            tc.tile_pool(name="psum", bufs=2, space="PSUM") as psum,
        ):
            x_sb = const.tile([SC, NV * 8], F32)
            nc.sync.dma_start(out=x_sb, in_=x_d[:].rearrange("s v c -> s (v c)"))
            ident = const.tile([128, 128], F32)
            nc.sync.dma_start(out=ident, in_=id_d[:])
            mu_tb = const.tile([128, NF, K], F32)
            _dma_bcast_rows(nc, mu_tb.rearrange("p i k -> p (i k)"), mu_t_d, NF * K)
            nictb = const.tile([128, NF, K], F32)
            _dma_bcast_rows(nc, nictb.rearrange("p i k -> p (i k)"), nict_d, NF * K)
            mu_rb = const.tile([128, NF, K], F32)
            _dma_bcast_rows(nc, mu_rb.rearrange("p i k -> p (i k)"), mu_r_d, NF * K)
            nicrb = const.tile([128, NF, K], F32)
            _dma_bcast_rows(nc, nicrb.rearrange("p i k -> p (i k)"), nicr_d, NF * K)
            w_sb = const.tile([K, NF, K], F32)
            nc.sync.dma_start(out=w_sb, in_=w_d[:].rearrange("i k l -> k i l"))
            b_sb = const.tile([K, NF], F32)
            nc.sync.dma_start(out=b_sb, in_=b_d[:].rearrange("i k -> k i"))

            xv = x_sb.rearrange("s (v c) -> s v c", c=8)
            featsT = const.tile([NV, NF, SC], F32)
            rhoT = const.tile([NV, SC], F32)
            thetaT = const.tile([NV, SC], F32)
            maskT = const.tile([NV, SC], F32)
            targets = [featsT[:, i, :] for i in range(NF)] + [rhoT, thetaT, maskT]
            for c in range(8):
                pt = psum.tile([NV, SC], F32, tag="tpsum")
                nc.tensor.transpose(pt, xv[:, :, c], ident[:SC, :SC])
                nc.scalar.copy(targets[c], pt)

            T = work.tile([NV, R, SC], F32)
            for r in range(R):
                nc.vector.tensor_scalar_add(T[:, r, :], thetaT, (TWO_PI / R) * r)
            ge = work.tile([NV, R, SC], F32)
            nc.vector.tensor_scalar(ge, T, TWO_PI, None, ALU.is_ge)
            nc.vector.scalar_tensor_tensor(T, ge, -TWO_PI, T, ALU.mult, ALU.add)

            mf = work.tile([NV, NF, SC], F32)
            nc.vector.tensor_mul(mf, featsT, _bcast(maskT, 1, [NV, NF, SC]))

            convmax = const.tile([K, NF, SC], F32)

            for i in range(NF):
                # arg_rho[v, s, k] for this feature
                argr = big.tile([NV, SC, K], F32, tag="argr")
                nc.vector.tensor_sub(
                    argr,
                    _bcast(rhoT, 2, [NV, SC, K]),
                    _bcast(mu_rb[:, i, :], 1, [NV, SC, K]),
                )
                nc.scalar.activation(argr, argr, AF.Square)
                nc.vector.tensor_mul(
                    argr, argr, _bcast(nicrb[:, i, :], 1, [NV, SC, K])
                )
                desc = big.tile([K, R * SC], F32, tag="desc")
                for r in range(R):
                    h = big.tile([NV, SC, K], F32, tag="h")
                    nc.vector.tensor_sub(
                        h,
                        _bcast(T[:, r, :], 2, [NV, SC, K]),
                        _bcast(mu_tb[:, i, :], 1, [NV, SC, K]),
                    )
                    nc.scalar.activation(h, h, AF.Square)
                    nc.vector.tensor_mul(
                        h, h, _bcast(nictb[:, i, :], 1, [NV, SC, K])
                    )
                    nc.vector.tensor_add(h, h, argr)
                    nc.scalar.activation(h, h, AF.Exp)
                    St = work.tile([NV, SC], F32, tag="St")
                    nc.vector.reduce_sum(St, h, axis=AX.X)
                    nc.vector.tensor_scalar_add(St, St, EPS)
                    rec = work.tile([NV, SC], F32, tag="rec")
                    nc.vector.reciprocal(rec, St)
                    wcol = work.tile([NV, SC], F32, tag="wcol")
                    nc.vector.tensor_mul(wcol, mf[:, i, :], rec)
                    dp = psum.tile([K, SC], F32, tag="dpsum")
                    for s in range(SC):
                        nc.tensor.matmul(
                            dp[:, s : s + 1],
                            h[:, s, :],
                            wcol[:, s : s + 1],
                            start=True, stop=True,
                        )
                    nc.scalar.copy(desc[:, r * SC : (r + 1) * SC], dp)

                # conv + max over rotations
                for half in range(2):
                    cp = psum.tile([K, R * SC // 2], F32, tag="cpsum")
                    nc.tensor.matmul(
                        cp,
                        w_sb[:, i, :],
                        desc[:, half * (R * SC // 2) : (half + 1) * (R * SC // 2)],
                        start=True, stop=True,
                    )
                    red = work.tile([K, SC], F32, tag="red")
                    nc.vector.reduce_max(
                        red, cp.rearrange("k (r s) -> k s r", r=R // 2), axis=AX.X
                    )
                    if half == 0:
                        nc.vector.tensor_copy(convmax[:, i, :], red)
                    else:
                        nc.vector.tensor_max(
                            convmax[:, i, :], convmax[:, i, :], red
                        )

            out_sb = const.tile([SC, K * NF], F32)
            oview = out_sb.rearrange("s (k i) -> s k i", i=NF)
            for i in range(NF):
                act = work.tile([K, SC], F32, tag="act")
                nc.scalar.activation(
                    act, convmax[:, i, :], AF.Relu, bias=b_sb[:, i : i + 1]
                )
                pt = psum.tile([SC, K], F32, tag="opsum")
                nc.tensor.transpose(pt, act, ident[:K, :K])
                nc.vector.tensor_copy(oview[:, :, i], pt)
            nc.sync.dma_start(out=out_d[:], in_=out_sb)

    return nc


# ---------------------------------------------------------------------------
# Host driver
# ---------------------------------------------------------------------------

def _detect_factored(mu_rho, sigma_rho, mu_theta, sigma_theta):
    k = np.arange(K)
    kt = k % NT
    kr = (k // NT) * NT
    for a in (mu_rho, sigma_rho, mu_theta, sigma_theta):
        if not np.all(a == a[0:1]):
            return None
    if not (np.array_equal(mu_theta, mu_theta[:, kt])
            and np.array_equal(sigma_theta, sigma_theta[:, kt])
            and np.array_equal(mu_rho, mu_rho[:, kr])
            and np.array_equal(sigma_rho, sigma_rho[:, kr])):
        return None
    c_t = sigma_theta[0, :NT].astype(np.float64) ** 2 + EPS
    c_r = sigma_rho[0, ::NT].astype(np.float64) ** 2 + EPS
    if not (np.all(c_t == c_t[0]) and np.all(c_r == c_r[0])):
        return None
    return {
        "mu_t": mu_theta[0, :NT].copy(),
        "mu_r": mu_rho[0, ::NT].copy(),
        "neg_inv_ct": -1.0 / c_t[0],
        "neg_inv_cr": -1.0 / c_r[0],
    }


_CACHE = {}


def _get_program(key, builder, *args):
    if key not in _CACHE:
        nc = builder(*args)
        if not nc.is_finalized():
            nc.finalize()
        _CACHE[key] = nc
    return _CACHE[key]


def prepare(inputs):
    """Build (or fetch cached) program and per-core input maps."""
    x = np.ascontiguousarray(inputs["x"], dtype=np.float32)
    mu_rho = np.asarray(inputs["mu_rho"], dtype=np.float32)
    sigma_rho = np.asarray(inputs["sigma_rho"], dtype=np.float32)
    mu_theta = np.asarray(inputs["mu_theta"], dtype=np.float32)
    sigma_theta = np.asarray(inputs["sigma_theta"], dtype=np.float32)
    W = np.ascontiguousarray(inputs["W_conv"], dtype=np.float32)
    b = np.ascontiguousarray(inputs["b_conv"], dtype=np.float32)

    ident = np.eye(128, dtype=np.float32)
    fact = _detect_factored(mu_rho, sigma_rho, mu_theta, sigma_theta)

    if fact is not None:
        nc = _get_program(
            ("fact", float(fact["neg_inv_ct"]), float(fact["neg_inv_cr"])),
            build_factored, fact["neg_inv_ct"], fact["neg_inv_cr"],
        )
        w_re = np.ascontiguousarray(
            W.reshape(NF, NR, NT, K).transpose(0, 2, 1, 3).reshape(NF, NT, NR * K)
        )
        common = {
            "mu_t": fact["mu_t"].reshape(1, NT),
            "mu_r": fact["mu_r"].reshape(1, NR),
            "w_re": w_re,
            "b_c": b,
            "ident": ident,
        }
    else:
        nc = _get_program(("gen",), build_generic)
        nict = (-1.0 / (sigma_theta.astype(np.float64) ** 2 + EPS)).astype(np.float32)
        nicr = (-1.0 / (sigma_rho.astype(np.float64) ** 2 + EPS)).astype(np.float32)
        common = {
            "mu_t": mu_theta.reshape(1, NF * K),
            "nict": nict.reshape(1, NF * K),
            "mu_r": mu_rho.reshape(1, NF * K),
            "nicr": nicr.reshape(1, NF * K),
            "w_c": W,
            "b_c": b,
            "ident": ident,
        }

    in_maps = []
    for c in range(N_CORES):
        m = dict(common)
        m["x"] = np.ascontiguousarray(x[c * SC : (c + 1) * SC])
        in_maps.append(m)
    return nc, in_maps


def kernel(**inputs):
    nc, in_maps = prepare(inputs)
    res = run_bass_kernel_spmd(nc, in_maps, core_ids=list(range(N_CORES)))
    return np.concatenate(
        [res.results[c]["out"] for c in range(N_CORES)], axis=0
    )
